# revision 1
# baseline (speedup 1.0000x reference)
"""Trainium2 Bass kernel for nn_AbstractAttention (B=2, S=2048, D=1024, H=16, dh=64).

Sharding: 8 cores = 2 batch groups x 4 cores. Core i handles batch i//4 and
heads 4*(i%4)..+4 for QKV projection + causal attention; z^T is AllGathered
(fp16, 1 MB/core) within each 4-core group and every core then runs the full
output projection; the host reads one core's output per batch.

Per core:
  - Q^T,K^T ([e,pos]) and V ([pos,e]) projections from DMA-transposed fp16
    inputs (host pre-casts to fp16; accumulation in fp32 PSUM).
  - per head: S^T[k,q] score tiles (only causal blocks), exp on ACT with the
    1/sqrt(dh) scale folded in, multiplicative triangular mask on diagonal
    blocks, unnormalized PV with a ones-column in V so z^T[64,:] is the
    softmax denominator; normalize via reciprocal + PE ones-broadcast.
  - AllGather z^T, out[2048,1024] = z_full^T.T @ W_O + b_O.
"""
import os, sys, types

sys.path.insert(0, "/opt/trn_rl_repo")
import numpy as np

import concourse.bass as bass
import concourse.bacc as bacc
import concourse.tile as tile
from concourse import mybir
from concourse.bass_utils import run_bass_kernel_spmd

B, S, D, H, DH = 2, 2048, 1024, 16, 64
N_CORES = 8
HPC = 4            # heads per core
QC = 512           # query chunk width for score tiles
NQC = S // QC      # 4
KB = 128           # key block
NKB = S // KB      # 16
NDMC = D // 128    # 8 d_model chunks
F16 = mybir.dt.float16
F32 = mybir.dt.float32
F32R = mybir.dt.float32r


def _install_ntff_hook():
    """Register the axon NTFF profiling hook missing from this image's antenv."""
    if "antenv.axon_hooks" in sys.modules:
        return
    try:
        from trn_agent_boot.trn_boot import _ntff_profile_via_ctypes

        hook = _ntff_profile_via_ctypes("/opt/axon/libaxon_pjrt.so")
        if hook is None:
            return
        import antenv  # noqa: F401

        mod = types.ModuleType("antenv.axon_hooks")
        mod.get_axon_ntff_profile_hook = lambda: hook
        sys.modules["antenv.axon_hooks"] = mod
    except Exception:
        pass


def build():
    nc = bacc.Bacc("TRN2", target_bir_lowering=False, debug=False, num_devices=N_CORES)
    xq = nc.dram_tensor("xq", [S, D], F16, kind="ExternalInput")
    xk = nc.dram_tensor("xk", [S, D], F16, kind="ExternalInput")
    xv = nc.dram_tensor("xv", [S, D], F16, kind="ExternalInput")
    wq = nc.dram_tensor("wq", [HPC, D, DH], F16, kind="ExternalInput")
    wk = nc.dram_tensor("wk", [HPC, D, DH], F16, kind="ExternalInput")
    wv = nc.dram_tensor("wv", [HPC, D, DH], F16, kind="ExternalInput")
    wo = nc.dram_tensor("wo", [H * DH, D], F16, kind="ExternalInput")
    bq = nc.dram_tensor("bq", [HPC, DH], F32, kind="ExternalInput")
    bk = nc.dram_tensor("bk", [HPC, DH], F32, kind="ExternalInput")
    bv = nc.dram_tensor("bv", [HPC, DH], F32, kind="ExternalInput")
    bo = nc.dram_tensor("bo", [D], F32, kind="ExternalInput")
    out = nc.dram_tensor("out", [S, D], F32, kind="ExternalOutput")

    tri_dram = nc.inline_tensor(np.triu(np.ones((128, 128), np.float16)), name="tri_c")
    ones_dram = nc.inline_tensor(np.ones((1, DH), np.float32), name="ones_c")

    with tile.TileContext(nc) as tc:
        with (
            tc.tile_pool(name="consts", bufs=1) as consts,
            tc.tile_pool(name="persist", bufs=1) as persist,
            tc.tile_pool(name="work", bufs=2) as work,
            tc.tile_pool(name="pt", bufs=3) as ptp,
            tc.tile_pool(name="zf", bufs=3) as zfp,
            tc.tile_pool(name="ps", bufs=1, space="PSUM") as ps,
            tc.tile_pool(name="ps2", bufs=2, space="PSUM") as ps2,
            tc.tile_pool(name="ps3", bufs=4, space="PSUM") as ps3,
            tc.tile_pool(name="dram", bufs=1, space="DRAM") as dram,
        ):
            # ---- constants / weights / biases -------------------------------
            tri = consts.tile([128, 128], F16, tag="tri")
            nc.sync.dma_start(out=tri, in_=tri_dram.ap())

            wq_sb = consts.tile([128, NDMC, HPC, DH], F16, tag="wq")
            wk_sb = consts.tile([128, NDMC, HPC, DH], F16, tag="wk")
            wv_sb = consts.tile([128, NDMC, HPC, DH], F16, tag="wv")
            for dmc in range(NDMC):
                sl = slice(128 * dmc, 128 * (dmc + 1))
                nc.sync.dma_start(
                    out=wq_sb[:, dmc], in_=wq.ap()[:, sl, :].rearrange("h d e -> d h e")
                )
                nc.sync.dma_start(
                    out=wk_sb[:, dmc], in_=wk.ap()[:, sl, :].rearrange("h d e -> d h e")
                )
                nc.sync.dma_start(
                    out=wv_sb[:, dmc], in_=wv.ap()[:, sl, :].rearrange("h d e -> d h e")
                )
            wo_sb = consts.tile([128, H * DH // 128, D], F16, tag="wo")
            nc.sync.dma_start(
                out=wo_sb, in_=wo.ap().rearrange("(c p) d -> p c d", p=128)
            )

            bq_sb = consts.tile([128, 2], F32, tag="bq")
            bk_sb = consts.tile([128, 2], F32, tag="bk")
            for hp in range(2):
                nc.gpsimd.dma_start(
                    out=bq_sb[:, hp : hp + 1],
                    in_=bass.AP(tensor=bq.ap().tensor, offset=128 * hp, ap=[[1, 128], [1, 1]]),
                )
                nc.gpsimd.dma_start(
                    out=bk_sb[:, hp : hp + 1],
                    in_=bass.AP(tensor=bk.ap().tensor, offset=128 * hp, ap=[[1, 128], [1, 1]]),
                )
            bv_sb = consts.tile([128, HPC, DH], F32, tag="bv")
            nc.gpsimd.dma_start(
                out=bv_sb,
                in_=bass.AP(tensor=bv.ap().tensor, offset=0, ap=[[0, 128], [64, HPC], [1, DH]]),
            )
            bo_sb = consts.tile([128, D], F32, tag="bo")
            nc.gpsimd.dma_start(
                out=bo_sb,
                in_=bass.AP(tensor=bo.ap().tensor, offset=0, ap=[[0, 128], [1, D]]),
            )
            ones32 = consts.tile([1, DH], F32, tag="ones32")
            nc.sync.dma_start(out=ones32, in_=ones_dram.ap())
            ones_r = consts.tile([1, DH], F32R, tag="ones")
            nc.vector.tensor_copy(ones_r, ones32)

            # ---- projections -------------------------------------------------
            qT = persist.tile([128, 2, S], F16, tag="qT")  # [2 heads stacked, hp, pos]
            kT = persist.tile([128, 2, S], F16, tag="kT")
            v_aug = persist.tile([128, NKB, HPC, DH + 1], F16, tag="vaug")
            nc.vector.memset(v_aug[:, :, :, DH : DH + 1], 1.0)
            zT_sb = persist.tile([128, 2, S], F16, tag="zT")

            def project(x_dram, w_sb, kind, b_sb):
                for pc in range(NQC):
                    xt = work.tile([128, NDMC, QC], F16, tag="xt")
                    for dmc in range(NDMC):
                        nc.sync.dma_start(
                            out=xt[:, dmc],
                            in_=x_dram.ap()[
                                QC * pc : QC * (pc + 1), 128 * dmc : 128 * (dmc + 1)
                            ],
                            transpose=True,
                        )
                    if kind in ("q", "k"):
                        dst = qT if kind == "q" else kT
                        for hp in range(2):
                            pj = ps.tile([128, QC], F32, tag="pj")
                            lhs = w_sb[:, :, 2 * hp : 2 * hp + 2, :]
                            for dmc in range(NDMC):
                                nc.tensor.matmul(
                                    pj,
                                    lhs[:, dmc].rearrange("p h e -> p (h e)"),
                                    xt[:, dmc],
                                    start=(dmc == 0),
                                    stop=(dmc == NDMC - 1),
                                )
                            nc.vector.tensor_scalar_add(
                                dst[:, hp, QC * pc : QC * (pc + 1)],
                                pj,
                                b_sb[:, hp : hp + 1],
                            )
                    else:
                        for pb4 in range(4):
                            pv_full = ps.tile([128, QC], F32, tag="pj")
                            pv = pv_full[:, 0 : HPC * DH]
                            for dmc in range(NDMC):
                                nc.tensor.matmul(
                                    pv,
                                    xt[:, dmc, 128 * pb4 : 128 * (pb4 + 1)],
                                    w_sb[:, dmc].rearrange("p h e -> p (h e)"),
                                    start=(dmc == 0),
                                    stop=(dmc == NDMC - 1),
                                )
                            kb = 4 * pc + pb4
                            nc.vector.tensor_add(
                                v_aug[:, kb, :, 0:DH],
                                pv.rearrange("p (h e) -> p h e", h=HPC),
                                bv_sb,
                            )

            project(xk, wk_sb, "k", bk_sb)
            project(xv, wv_sb, "v", None)
            project(xq, wq_sb, "q", bq_sb)

            # ---- attention + incremental allgather --------------------------
            z_dram = dram.tile([4, 64, S], F16, tag="zd")
            zfh0 = dram.tile([4, 64, S], F16, tag="zfh0")
            zfh1 = dram.tile([4, 64, S], F16, tag="zfh1")
            zfh2 = dram.tile([4, 64, S], F16, tag="zfh2")
            zfh3 = dram.tile([4, 64, S], F16, tag="zfh3")
            zfh = [zfh0, zfh1, zfh2, zfh3]
            for h in range(HPC):
                for qc in range(NQC):
                    hp, m0 = h // 2, 64 * (h % 2)
                    zps = ps2.tile([DH + 1, QC], F32, tag="zps")
                    for kb in range(4 * qc + 4):
                        m = kb - 4 * qc
                        off = 0 if m < 0 else 128 * m
                        w = QC - off
                        st = ps3.tile([128, QC], F32, tag="st")
                        nc.tensor.matmul(
                            st[:, 0:w],
                            kT[m0 : m0 + 64, hp, 128 * kb : 128 * (kb + 1)],
                            qT[m0 : m0 + 64, hp, QC * qc + off : QC * (qc + 1)],
                            start=True,
                            stop=True,
                        )
                        pt = ptp.tile([128, QC], F16, tag="pt")
                        nc.scalar.activation(
                            pt[:, 0:w],
                            st[:, 0:w],
                            mybir.ActivationFunctionType.Exp,
                            scale=0.125,
                        )
                        if m >= 0:
                            nc.vector.tensor_mul(pt[:, 0:128], pt[:, 0:128], tri)
                        nc.tensor.matmul(
                            zps[:, off:QC],
                            v_aug[:, kb, h],
                            pt[:, 0:w],
                            start=(kb == 0),
                            stop=(kb == 4 * qc + 3),
                        )
                    rec32 = work.tile([1, QC], F32, tag="rec32")
                    nc.vector.reciprocal(rec32, zps[DH : DH + 1, :])
                    rec = work.tile([1, QC], F32R, tag="rec")
                    with nc.allow_low_precision(reason="f32r holds full fp32 bits"):
                        nc.vector.tensor_copy(rec, rec32)
                    bc = ps.tile([DH, QC], F32, tag="bc")
                    nc.tensor.matmul(bc, ones_r, rec, start=True, stop=True)
                    bcs = work.tile([DH, QC], F32, tag="bcs")
                    nc.scalar.copy(bcs, bc)
                    nc.vector.tensor_mul(
                        zT_sb[m0 : m0 + 64, hp, QC * qc : QC * (qc + 1)],
                        zps[0:DH, :],
                        bcs,
                    )

                m0h = 64 * (h % 2)
                nc.sync.dma_start(out=z_dram[h], in_=zT_sb[m0h : m0h + 64, h // 2, :])
                nc.gpsimd.collective_compute(
                    "AllGather",
                    mybir.AluOpType.bypass,
                    replica_groups=[[0, 1, 2, 3], [4, 5, 6, 7]],
                    ins=[z_dram[h].opt()],
                    outs=[zfh[h].opt()],
                )

            zfull_sb = persist.tile([128, 8, S], F16, tag="zfull")
            for c in range(8):
                j, p = c // 2, c % 2
                nc.sync.dma_start(out=zfull_sb[0:64, c], in_=zfh[2 * p][j])
                nc.sync.dma_start(out=zfull_sb[64:128, c], in_=zfh[2 * p + 1][j])
            for qb in range(S // 128):
                po0 = ps3.tile([128, 512], F32, tag="st")
                po1 = ps3.tile([128, 512], F32, tag="st")
                pos = (po0, po1)
                ob = work.tile([128, D], F32, tag="ob")
                for c in range(8):
                    for dh2 in range(2):
                        nc.tensor.matmul(
                            pos[dh2],
                            zfull_sb[:, c, 128 * qb : 128 * (qb + 1)],
                            wo_sb[:, c, 512 * dh2 : 512 * (dh2 + 1)],
                            start=(c == 0),
                            stop=(c == 7),
                        )
                for dh2 in range(2):
                    nc.vector.tensor_add(
                        ob[:, 512 * dh2 : 512 * (dh2 + 1)],
                        pos[dh2],
                        bo_sb[:, 512 * dh2 : 512 * (dh2 + 1)],
                    )
                nc.sync.dma_start(out=out.ap()[128 * qb : 128 * (qb + 1), :], in_=ob)

    nc.finalize()
    return nc


_CACHE = {}


def kernel(**inputs):
    _install_ntff_hook()
    nc = _CACHE.get("nc")
    if nc is None:
        nc = build()
        _CACHE["nc"] = nc

    f16 = np.float16
    xs = {k: np.asarray(inputs[k], np.float32) for k in ("query_input", "key_input", "value_input")}
    W = {k: np.asarray(inputs[k], np.float32) for k in ("W_Q", "W_K", "W_V", "W_O")}
    b = {k: np.asarray(inputs[k], np.float32) for k in ("b_Q", "b_K", "b_V", "b_O")}
    x16 = {k: np.ascontiguousarray(v).astype(f16) for k, v in xs.items()}
    wo16 = np.ascontiguousarray(W["W_O"].reshape(H * DH, D)).astype(f16)

    in_maps = []
    for i in range(N_CORES):
        g, h0 = i // 4, 4 * (i % 4)
        in_maps.append(
            {
                "xq": x16["query_input"][g],
                "xk": x16["key_input"][g],
                "xv": x16["value_input"][g],
                "wq": np.ascontiguousarray(W["W_Q"][h0 : h0 + HPC]).astype(f16),
                "wk": np.ascontiguousarray(W["W_K"][h0 : h0 + HPC]).astype(f16),
                "wv": np.ascontiguousarray(W["W_V"][h0 : h0 + HPC]).astype(f16),
                "wo": wo16,
                "bq": np.ascontiguousarray(b["b_Q"][h0 : h0 + HPC]),
                "bk": np.ascontiguousarray(b["b_K"][h0 : h0 + HPC]),
                "bv": np.ascontiguousarray(b["b_V"][h0 : h0 + HPC]),
                "bo": np.ascontiguousarray(b["b_O"]),
            }
        )

    res = run_bass_kernel_spmd(nc, in_maps, core_ids=list(range(N_CORES)))
    if os.environ.get("KERNEL_PRINT_EXEC"):
        print(f"HW exec time: {res.exec_time_ns} ns")
    return np.stack([res.results[0]["out"], res.results[4]["out"]], axis=0).astype(np.float32)



# revision 6
# speedup vs baseline: 1.4122x; 1.4122x over previous
"""Trainium2 Bass kernel for nn_AbstractAttention (B=2, S=2048, D=1024, H=16, dh=64).

Sharding: 8 cores = 2 batch groups x 4 cores. Core i handles batch i//4 and
heads 4*(i%4)..+4 for QKV projection + causal attention; z^T is AllGathered
(fp16) within each 4-core group and every core then runs the output projection
for its own 256-column slice of d_model (host slices W_O per core); the host
concatenates the 4 column slices per batch.

v2 changes vs baseline:
  - host pre-transposes x to [D, S] and pre-arranges W to [D, he] so every
    DMA is a plain contiguous load (no serialized DMA-transpose).
  - v_aug carries 64 ones-columns so the PV matmul broadcasts the softmax
    denominator across partitions 64:128 of zps; normalization is a batched
    64-lane reciprocal_approx_fast + one multiply (no 1-lane reciprocal, no
    PE broadcast matmul).
  - W_O column sharding removes the 4x-replicated output projection.
  - score/exp/PV loop is software-pipelined with a 4-deep score lead so the
    PE never serializes behind the Scalar engine's exp.
"""
import os, sys, types

sys.path.insert(0, "/opt/trn_rl_repo")
import numpy as np

import concourse.bass as bass
import concourse.bacc as bacc
import concourse.tile as tile
from concourse import mybir
from concourse.bass_utils import run_bass_kernel_spmd

B, S, D, H, DH = 2, 2048, 1024, 16, 64
N_CORES = 8
HPC = 4            # heads per core
QC = 512           # query chunk width for score tiles
NQC = S // QC      # 4
KB = 128           # key block
NKB = S // KB      # 16
NDMC = D // 128    # 8 d_model chunks
DO = D // 4        # out-projection d_model columns per core
LEAD = 4           # score tiles in flight ahead of PV
F16 = mybir.dt.float16
F32 = mybir.dt.float32


def _install_ntff_hook():
    """Register the axon NTFF profiling hook missing from this image's antenv."""
    if "antenv.axon_hooks" in sys.modules:
        return
    try:
        from trn_agent_boot.trn_boot import _ntff_profile_via_ctypes

        hook = _ntff_profile_via_ctypes("/opt/axon/libaxon_pjrt.so")
        if hook is None:
            return
        import antenv  # noqa: F401

        mod = types.ModuleType("antenv.axon_hooks")
        mod.get_axon_ntff_profile_hook = lambda: hook
        sys.modules["antenv.axon_hooks"] = mod
    except Exception:
        pass


def build():
    nc = bacc.Bacc("TRN2", target_bir_lowering=False, debug=False, num_devices=N_CORES)
    xq = nc.dram_tensor("xq", [D, S], F16, kind="ExternalInput")
    xk = nc.dram_tensor("xk", [D, S], F16, kind="ExternalInput")
    xv = nc.dram_tensor("xv", [D, S], F16, kind="ExternalInput")
    wq = nc.dram_tensor("wq", [D, HPC * DH], F16, kind="ExternalInput")
    wk = nc.dram_tensor("wk", [D, HPC * DH], F16, kind="ExternalInput")
    wv = nc.dram_tensor("wv", [D, HPC * DH], F16, kind="ExternalInput")
    wo = nc.dram_tensor("wo", [H * DH, DO], F16, kind="ExternalInput")
    bq = nc.dram_tensor("bq", [HPC, DH], F32, kind="ExternalInput")
    bk = nc.dram_tensor("bk", [HPC, DH], F32, kind="ExternalInput")
    bv = nc.dram_tensor("bv", [HPC, DH], F32, kind="ExternalInput")
    bo = nc.dram_tensor("bo", [DO], F32, kind="ExternalInput")
    out = nc.dram_tensor("out", [S, DO], F32, kind="ExternalOutput")

    tri_dram = nc.inline_tensor(np.triu(np.ones((128, 128), np.float16)), name="tri_c")

    with tile.TileContext(nc) as tc:
        with (
            tc.tile_pool(name="consts", bufs=1) as consts,
            tc.tile_pool(name="persist", bufs=1) as persist,
            tc.tile_pool(name="xpool", bufs=2) as xpool,
            tc.tile_pool(name="ptp", bufs=4) as ptp,
            tc.tile_pool(name="recp", bufs=2) as recp,
            tc.tile_pool(name="obp", bufs=2) as obp,
            tc.tile_pool(name="psA", bufs=2, space="PSUM") as psA,
            tc.tile_pool(name="psB", bufs=4, space="PSUM") as psB,
            tc.tile_pool(name="psC", bufs=2, space="PSUM") as psC,
            tc.tile_pool(name="dram", bufs=1, space="DRAM") as dram,
        ):
            # ---- constants / weights / biases (gpsimd queue) ----------------
            tri = consts.tile([128, 128], F16, tag="tri")
            nc.gpsimd.dma_start(out=tri, in_=tri_dram.ap())

            wq_sb = consts.tile([128, NDMC, HPC * DH], F16, tag="wq")
            wk_sb = consts.tile([128, NDMC, HPC * DH], F16, tag="wk")
            wv_sb = consts.tile([128, NDMC, HPC * DH], F16, tag="wv")
            nc.gpsimd.dma_start(out=wq_sb, in_=wq.ap().rearrange("(c p) e -> p c e", p=128))
            nc.gpsimd.dma_start(out=wk_sb, in_=wk.ap().rearrange("(c p) e -> p c e", p=128))
            nc.gpsimd.dma_start(out=wv_sb, in_=wv.ap().rearrange("(c p) e -> p c e", p=128))
            wo_sb = consts.tile([128, H * DH // 128, DO], F16, tag="wo")
            nc.gpsimd.dma_start(out=wo_sb, in_=wo.ap().rearrange("(c p) d -> p c d", p=128))

            bq_sb = consts.tile([128, 2], F32, tag="bq")
            bk_sb = consts.tile([128, 2], F32, tag="bk")
            for hp in range(2):
                nc.gpsimd.dma_start(
                    out=bq_sb[:, hp : hp + 1],
                    in_=bass.AP(tensor=bq.ap().tensor, offset=128 * hp, ap=[[1, 128], [1, 1]]),
                )
                nc.gpsimd.dma_start(
                    out=bk_sb[:, hp : hp + 1],
                    in_=bass.AP(tensor=bk.ap().tensor, offset=128 * hp, ap=[[1, 128], [1, 1]]),
                )
            bv_sb = consts.tile([128, HPC, DH], F32, tag="bv")
            nc.gpsimd.dma_start(
                out=bv_sb,
                in_=bass.AP(tensor=bv.ap().tensor, offset=0, ap=[[0, 128], [64, HPC], [1, DH]]),
            )
            bo_sb = consts.tile([128, DO], F32, tag="bo")
            nc.gpsimd.dma_start(
                out=bo_sb,
                in_=bass.AP(tensor=bo.ap().tensor, offset=0, ap=[[0, 128], [1, DO]]),
            )

            # ---- persistent activation tiles --------------------------------
            kT = persist.tile([128, 2, S], F16, tag="kT")   # [2 heads stacked, hp, pos]
            qTs = [
                persist.tile([128, 2, QC], F16, tag=f"qT{pc}", name=f"qT{pc}")
                for pc in range(NQC)
            ]
            v_aug = persist.tile([128, NKB, HPC, 2 * DH], F16, tag="vaug")
            nc.vector.memset(v_aug[:, :, :, DH : 2 * DH], 1.0)
            zTs = [
                persist.tile([64, S], F16, tag=f"zT{h}", name=f"zT{h}")
                for h in range(HPC)
            ]
            zfull_sb = persist.tile([128, 8, S], F16, tag="zfull")

            # ---- projections -------------------------------------------------
            def load_half(x_dram, hf):
                t = xpool.tile([128, NDMC, S // 2], F16, tag="xh")
                for dmc in range(NDMC):
                    nc.sync.dma_start(
                        out=t[:, dmc],
                        in_=x_dram.ap()[
                            128 * dmc : 128 * (dmc + 1),
                            (S // 2) * hf : (S // 2) * (hf + 1),
                        ],
                    )
                return t

            def proj_qk(dst_pc, dst_sl, w_sb, b_sb, t, pcl):
                for hp in range(2):
                    pj = psA.tile([128, 512], F32, tag="pj")
                    for dmc in range(NDMC):
                        nc.tensor.matmul(
                            pj,
                            w_sb[:, dmc, 128 * hp : 128 * (hp + 1)],
                            t[:, dmc, QC * pcl : QC * (pcl + 1)],
                            start=(dmc == 0),
                            stop=(dmc == NDMC - 1),
                        )
                    nc.vector.tensor_scalar_add(
                        dst_pc[:, hp, dst_sl], pj, b_sb[:, hp : hp + 1]
                    )

            def proj_v(t, pcl, pc):
                for pb4 in range(4):
                    pv_full = psA.tile([128, 512], F32, tag="pj")
                    pv = pv_full[:, 0 : HPC * DH]
                    for dmc in range(NDMC):
                        nc.tensor.matmul(
                            pv,
                            t[:, dmc, QC * pcl + 128 * pb4 : QC * pcl + 128 * (pb4 + 1)],
                            wv_sb[:, dmc],
                            start=(dmc == 0),
                            stop=(dmc == NDMC - 1),
                        )
                    kb = 4 * pc + pb4
                    nc.vector.tensor_add(
                        v_aug[:, kb, :, 0:DH],
                        pv.rearrange("p (h e) -> p h e", h=HPC),
                        bv_sb,
                    )

            # K then V then Q chunk 0; remaining Q chunks interleave with head 0.
            tk = [load_half(xk, 0), load_half(xk, 1)]
            for pc in range(NQC):
                proj_qk(kT, slice(QC * pc, QC * (pc + 1)), wk_sb, bk_sb, tk[pc // 2], pc % 2)
            tv = [load_half(xv, 0), load_half(xv, 1)]
            for pc in range(NQC):
                proj_v(tv[pc // 2], pc % 2, pc)
            tq = [load_half(xq, 0), load_half(xq, 1)]

            def proj_q(pc):
                proj_qk(qTs[pc], slice(0, QC), wq_sb, bq_sb, tq[pc // 2], pc % 2)

            proj_q(0)

            # ---- attention + incremental allgather --------------------------
            z_dram = dram.tile([HPC, 64, S], F16, tag="zd")
            zfh = [
                dram.tile([4, 64, S], F16, tag=f"zfh{h}", name=f"zfh{h}")
                for h in range(HPC)
            ]

            def attention(h, qc):
                hp, m0 = h // 2, 64 * (h % 2)
                nblk = 4 * qc + 4
                zps = psC.tile([128, QC], F32, tag="zps")
                pts = {}

                def emit_score(kb):
                    m = kb - 4 * qc
                    off = 0 if m < 0 else 128 * m
                    w = QC - off
                    st = psB.tile([128, QC], F32, tag="st")
                    nc.tensor.matmul(
                        st[:, 0:w],
                        kT[m0 : m0 + 64, hp, 128 * kb : 128 * (kb + 1)],
                        qTs[qc][m0 : m0 + 64, hp, off:QC],
                        start=True,
                        stop=True,
                    )
                    pt = ptp.tile([128, QC], F16, tag="pt")
                    nc.scalar.activation(
                        pt[:, 0:w],
                        st[:, 0:w],
                        mybir.ActivationFunctionType.Exp,
                        scale=0.125,
                    )
                    if m >= 0:
                        nc.vector.tensor_mul(pt[:, 0:128], pt[:, 0:128], tri)
                    pts[kb] = (pt, off, w)

                def emit_pv(kb):
                    pt, off, w = pts.pop(kb)
                    nc.tensor.matmul(
                        zps[:, off:QC],
                        v_aug[:, kb, h],
                        pt[:, 0:w],
                        start=(kb == 0),
                        stop=(kb == nblk - 1),
                    )

                for kb in range(min(LEAD, nblk)):
                    emit_score(kb)
                for kb in range(nblk):
                    emit_pv(kb)
                    if kb + LEAD < nblk:
                        emit_score(kb + LEAD)

                rec = recp.tile([64, QC], F32, tag="rec")
                nc.vector.reciprocal(out=rec, in_=zps[64:128, :])
                nc.vector.tensor_mul(
                    zTs[h][:, QC * qc : QC * (qc + 1)], zps[0:64, :], rec
                )

            for h in range(HPC):
                for qc in range(NQC):
                    attention(h, qc)
                    if h == 0 and qc + 1 < NQC:
                        proj_q(qc + 1)
                nc.sync.dma_start(out=z_dram[h], in_=zTs[h])
                nc.gpsimd.collective_compute(
                    "AllGather",
                    mybir.AluOpType.bypass,
                    replica_groups=[[0, 1, 2, 3], [4, 5, 6, 7]],
                    ins=[z_dram[h].opt()],
                    outs=[zfh[h].opt()],
                )
                m0h = 64 * (h % 2)
                for j in range(4):
                    nc.gpsimd.dma_start(
                        out=zfull_sb[m0h : m0h + 64, 2 * j + h // 2], in_=zfh[h][j]
                    )

            # ---- output projection (own DO-column slice of W_O) -------------
            for qb in range(S // 128):
                po_full = psA.tile([128, 512], F32, tag="pj")
                po = po_full[:, 0:DO]
                for c in range(8):
                    nc.tensor.matmul(
                        po,
                        zfull_sb[:, c, 128 * qb : 128 * (qb + 1)],
                        wo_sb[:, c],
                        start=(c == 0),
                        stop=(c == 7),
                    )
                ob = obp.tile([128, DO], F32, tag="ob")
                nc.vector.tensor_add(ob, po, bo_sb)
                nc.sync.dma_start(out=out.ap()[128 * qb : 128 * (qb + 1), :], in_=ob)

    nc.finalize()
    return nc


_CACHE = {}


def kernel(**inputs):
    _install_ntff_hook()
    nc = _CACHE.get("nc")
    if nc is None:
        nc = build()
        _CACHE["nc"] = nc

    f16 = np.float16
    xs = {k: np.asarray(inputs[k], np.float32) for k in ("query_input", "key_input", "value_input")}
    W = {k: np.asarray(inputs[k], np.float32) for k in ("W_Q", "W_K", "W_V", "W_O")}
    b = {k: np.asarray(inputs[k], np.float32) for k in ("b_Q", "b_K", "b_V", "b_O")}
    # pre-transpose activations to [D, S] so device DMAs are contiguous
    xT16 = {k: [np.ascontiguousarray(v[g].T).astype(f16) for g in range(B)] for k, v in xs.items()}
    # pre-arrange projection weights to [D, he] per head group
    wd = {}
    for k in ("W_Q", "W_K", "W_V"):
        wd[k] = [
            np.ascontiguousarray(
                W[k][4 * r : 4 * (r + 1)].transpose(1, 0, 2).reshape(D, HPC * DH)
            ).astype(f16)
            for r in range(4)
        ]
    wo_full = W["W_O"].reshape(H * DH, D)
    wo_slices = [np.ascontiguousarray(wo_full[:, DO * r : DO * (r + 1)]).astype(f16) for r in range(4)]
    bo_slices = [np.ascontiguousarray(b["b_O"][DO * r : DO * (r + 1)]) for r in range(4)]

    in_maps = []
    for i in range(N_CORES):
        g, r = i // 4, i % 4
        in_maps.append(
            {
                "xq": xT16["query_input"][g],
                "xk": xT16["key_input"][g],
                "xv": xT16["value_input"][g],
                "wq": wd["W_Q"][r],
                "wk": wd["W_K"][r],
                "wv": wd["W_V"][r],
                "wo": wo_slices[r],
                "bq": np.ascontiguousarray(b["b_Q"][4 * r : 4 * (r + 1)]),
                "bk": np.ascontiguousarray(b["b_K"][4 * r : 4 * (r + 1)]),
                "bv": np.ascontiguousarray(b["b_V"][4 * r : 4 * (r + 1)]),
                "bo": bo_slices[r],
            }
        )

    res = run_bass_kernel_spmd(nc, in_maps, core_ids=list(range(N_CORES)))
    if os.environ.get("KERNEL_PRINT_EXEC"):
        print(f"HW exec time: {res.exec_time_ns} ns")
    outs = []
    for g in range(B):
        outs.append(
            np.concatenate([res.results[4 * g + r]["out"] for r in range(4)], axis=1)
        )
    return np.stack(outs, axis=0).astype(np.float32)


# revision 8
# speedup vs baseline: 1.8503x; 1.3103x over previous
"""Trainium2 Bass kernel for nn_AbstractAttention (B=2, S=2048, D=1024, H=16, dh=64).

Sharding: 8 cores = 2 batch groups x 4 cores. Core i handles batch i//4 and
heads 4*(i%4)..+4 for QKV projection + causal attention; z^T is AllGathered
(fp16) within each 4-core group and every core then runs the output projection
for its own 256-column slice of d_model (host slices W_O per core); the host
concatenates the 4 column slices per batch.

Key structure (v3):
  - host pre-transposes x to [D, S] and pre-arranges W to [D, he]: all DMAs
    are contiguous loads, one per 128-row d_model chunk for fine-grained deps.
  - v_aug carries 64 ones-columns so the PV matmul broadcasts the softmax
    denominator into partitions 64:128 of zps for free.
  - normalization = fast-inverse bit trick + 1 Newton step (4 cheap DVE ops)
    instead of the 8-cycle/elem iterative-divide RECIPROCAL.
  - full (off-diagonal) score blocks are exp'd in 1024-wide pairs to halve
    Scalar-engine instruction/semaphore overhead.
  - V and Q projections are interleaved into head 0's attention stream.
  - output projection runs in two passes (even chunks after head 1's
    AllGather, odd after head 3's) to shorten the tail.
"""
import os, sys, types

sys.path.insert(0, "/opt/trn_rl_repo")
import numpy as np

import concourse.bass as bass
import concourse.bacc as bacc
import concourse.tile as tile
from concourse import mybir
from concourse.bass_utils import run_bass_kernel_spmd

B, S, D, H, DH = 2, 2048, 1024, 16, 64
N_CORES = 8
HPC = 4            # heads per core
QC = 512           # query chunk width for score tiles
NQC = S // QC      # 4
KB = 128           # key block
NKB = S // KB      # 16
NDMC = D // 128    # 8 d_model chunks
DO = D // 4        # out-projection d_model columns per core
LEADU = 2          # score units in flight ahead of PV
MAGIC = 0x7EF311C3  # fast-inverse-reciprocal seed constant
F16 = mybir.dt.float16
F32 = mybir.dt.float32
I32 = mybir.dt.int32


def _install_ntff_hook():
    """Register the axon NTFF profiling hook missing from this image's antenv."""
    if "antenv.axon_hooks" in sys.modules:
        return
    try:
        from trn_agent_boot.trn_boot import _ntff_profile_via_ctypes

        hook = _ntff_profile_via_ctypes("/opt/axon/libaxon_pjrt.so")
        if hook is None:
            return
        import antenv  # noqa: F401

        mod = types.ModuleType("antenv.axon_hooks")
        mod.get_axon_ntff_profile_hook = lambda: hook
        sys.modules["antenv.axon_hooks"] = mod
    except Exception:
        pass


def build():
    nc = bacc.Bacc("TRN2", target_bir_lowering=False, debug=False, num_devices=N_CORES)
    xq = nc.dram_tensor("xq", [D, S], F16, kind="ExternalInput")
    xk = nc.dram_tensor("xk", [D, S], F16, kind="ExternalInput")
    xv = nc.dram_tensor("xv", [D, S], F16, kind="ExternalInput")
    wq = nc.dram_tensor("wq", [D, HPC * DH], F16, kind="ExternalInput")
    wk = nc.dram_tensor("wk", [D, HPC * DH], F16, kind="ExternalInput")
    wv = nc.dram_tensor("wv", [D, HPC * DH], F16, kind="ExternalInput")
    wo = nc.dram_tensor("wo", [H * DH, DO], F16, kind="ExternalInput")
    bq = nc.dram_tensor("bq", [HPC, DH], F32, kind="ExternalInput")
    bk = nc.dram_tensor("bk", [HPC, DH], F32, kind="ExternalInput")
    bv = nc.dram_tensor("bv", [HPC, DH], F32, kind="ExternalInput")
    bo = nc.dram_tensor("bo", [DO], F32, kind="ExternalInput")
    out = nc.dram_tensor("out", [S, DO], F32, kind="ExternalOutput")

    tri_dram = nc.inline_tensor(np.triu(np.ones((128, 128), np.float16)), name="tri_c")

    with tile.TileContext(nc) as tc:
        with (
            tc.tile_pool(name="consts", bufs=1) as consts,
            tc.tile_pool(name="persist", bufs=1) as persist,
            tc.tile_pool(name="xpool", bufs=3) as xpool,
            tc.tile_pool(name="ptp", bufs=3) as ptp,
            tc.tile_pool(name="recp", bufs=2) as recp,
            tc.tile_pool(name="obp", bufs=2) as obp,
            tc.tile_pool(name="psB", bufs=3, space="PSUM") as psB,
            tc.tile_pool(name="psC", bufs=2, space="PSUM") as psC,
            tc.tile_pool(name="dram", bufs=1, space="DRAM") as dram,
        ):
            # ---- constants / weights / biases (gpsimd queue; K's first) -----
            wk_sb = consts.tile([128, NDMC, HPC * DH], F16, tag="wk")
            nc.gpsimd.dma_start(out=wk_sb, in_=wk.ap().rearrange("(c p) e -> p c e", p=128))
            bk_sb = consts.tile([128, 2], F32, tag="bk")
            bq_sb = consts.tile([128, 2], F32, tag="bq")
            for hp in range(2):
                nc.gpsimd.dma_start(
                    out=bk_sb[:, hp : hp + 1],
                    in_=bass.AP(tensor=bk.ap().tensor, offset=128 * hp, ap=[[1, 128], [1, 1]]),
                )
                nc.gpsimd.dma_start(
                    out=bq_sb[:, hp : hp + 1],
                    in_=bass.AP(tensor=bq.ap().tensor, offset=128 * hp, ap=[[1, 128], [1, 1]]),
                )
            wv_sb = consts.tile([128, NDMC, HPC * DH], F16, tag="wv")
            nc.gpsimd.dma_start(out=wv_sb, in_=wv.ap().rearrange("(c p) e -> p c e", p=128))
            bv_sb = consts.tile([128, HPC, DH], F32, tag="bv")
            nc.gpsimd.dma_start(
                out=bv_sb,
                in_=bass.AP(tensor=bv.ap().tensor, offset=0, ap=[[0, 128], [64, HPC], [1, DH]]),
            )
            wq_sb = consts.tile([128, NDMC, HPC * DH], F16, tag="wq")
            nc.gpsimd.dma_start(out=wq_sb, in_=wq.ap().rearrange("(c p) e -> p c e", p=128))
            tri = consts.tile([128, 128], F16, tag="tri")
            nc.gpsimd.dma_start(out=tri, in_=tri_dram.ap())
            wo_sb = consts.tile([128, H * DH // 128, DO], F16, tag="wo")
            nc.gpsimd.dma_start(out=wo_sb, in_=wo.ap().rearrange("(c p) d -> p c d", p=128))
            bo_sb = consts.tile([128, DO], F32, tag="bo")
            nc.gpsimd.dma_start(
                out=bo_sb,
                in_=bass.AP(tensor=bo.ap().tensor, offset=0, ap=[[0, 128], [1, DO]]),
            )
            magic_sb = consts.tile([64, QC], I32, tag="magic")
            nc.vector.memset(magic_sb, MAGIC)

            # ---- persistent activation tiles --------------------------------
            kT = persist.tile([128, 2, S], F16, tag="kT")   # [2 heads stacked, hp, pos]
            qTs = [
                persist.tile([128, 2, QC], F16, tag=f"qT{pc}", name=f"qT{pc}")
                for pc in range(NQC)
            ]
            v_aug = persist.tile([128, NKB, HPC, 2 * DH], F16, tag="vaug")
            nc.vector.memset(v_aug[:, :, :, DH : 2 * DH], 1.0)
            zTs = [
                persist.tile([64, S], F16, tag=f"zT{h}", name=f"zT{h}")
                for h in range(HPC)
            ]
            zfull_sb = persist.tile([128, 8, S], F16, tag="zfull")
            ob_stage = persist.tile([128, S // 128, DO], F32, tag="obst")

            # ---- projections -------------------------------------------------
            def load_half(x_dram, hf, pfx):
                ts = []
                for dmc in range(NDMC):
                    t = xpool.tile(
                        [128, S // 2], F16, tag=f"xh{dmc}", name=f"{pfx}{hf}_{dmc}"
                    )
                    nc.sync.dma_start(
                        out=t,
                        in_=x_dram.ap()[
                            128 * dmc : 128 * (dmc + 1),
                            (S // 2) * hf : (S // 2) * (hf + 1),
                        ],
                    )
                    ts.append(t)
                return ts

            def proj_qk(dst_pc, dst_sl, w_sb, b_sb, t, pcl):
                for hp in range(2):
                    pj_full = psB.tile([128, 1024], F32, tag="st")
                    pj = pj_full[:, 0:512]
                    for dmc in range(NDMC):
                        nc.tensor.matmul(
                            pj,
                            w_sb[:, dmc, 128 * hp : 128 * (hp + 1)],
                            t[dmc][:, QC * pcl : QC * (pcl + 1)],
                            start=(dmc == 0),
                            stop=(dmc == NDMC - 1),
                        )
                    nc.vector.tensor_scalar_add(
                        dst_pc[:, hp, dst_sl], pj, b_sb[:, hp : hp + 1]
                    )

            def proj_v(t, pcl, pc):
                for pb4 in range(4):
                    pv_full = psB.tile([128, 1024], F32, tag="st")
                    pv = pv_full[:, 0 : HPC * DH]
                    for dmc in range(NDMC):
                        nc.tensor.matmul(
                            pv,
                            t[dmc][:, QC * pcl + 128 * pb4 : QC * pcl + 128 * (pb4 + 1)],
                            wv_sb[:, dmc],
                            start=(dmc == 0),
                            stop=(dmc == NDMC - 1),
                        )
                    kb = 4 * pc + pb4
                    nc.vector.tensor_add(
                        v_aug[:, kb, :, 0:DH],
                        pv.rearrange("p (h e) -> p h e", h=HPC),
                        bv_sb,
                    )

            tk = [load_half(xk, 0, "xk"), load_half(xk, 1, "xk")]
            for pc in range(NQC):
                proj_qk(kT, slice(QC * pc, QC * (pc + 1)), wk_sb, bk_sb, tk[pc // 2], pc % 2)
            tv = [load_half(xv, 0, "xv"), load_half(xv, 1, "xv")]
            tq = [load_half(xq, 0, "xq"), load_half(xq, 1, "xq")]

            def proj_q(pc):
                proj_qk(qTs[pc], slice(0, QC), wq_sb, bq_sb, tq[pc // 2], pc % 2)

            # ---- attention ---------------------------------------------------
            z_dram = dram.tile([HPC, 64, S], F16, tag="zd")
            zfh = [
                dram.tile([4, 64, S], F16, tag=f"zfh{h}", name=f"zfh{h}")
                for h in range(HPC)
            ]

            def attention(h, qc):
                hp, m0 = h // 2, 64 * (h % 2)
                nblk = 4 * qc + 4
                # units: pairs of full blocks, then single diagonal blocks
                units = []
                for kb in range(0, 4 * qc, 2):
                    units.append([(kb, 0, 0, 512), (kb + 1, 512, 0, 512)])
                for m in range(4):
                    units.append([(4 * qc + m, 0, 128 * m, 512 - 128 * m)])
                zps = psC.tile([128, QC], F32, tag="zps")
                state = {}

                def emit_scores(ui):
                    unit = units[ui]
                    st = psB.tile([128, 1024], F32, tag="st")
                    for kb, co, off, w in unit:
                        nc.tensor.matmul(
                            st[:, co : co + w],
                            kT[m0 : m0 + 64, hp, 128 * kb : 128 * (kb + 1)],
                            qTs[qc][m0 : m0 + 64, hp, off:QC],
                            start=True,
                            stop=True,
                        )
                    pt = ptp.tile([128, 1024], F16, tag="pt")
                    tw = unit[-1][1] + unit[-1][3]
                    nc.scalar.activation(
                        pt[:, 0:tw],
                        st[:, 0:tw],
                        mybir.ActivationFunctionType.Exp,
                        scale=0.125,
                    )
                    if unit[0][2] or len(unit) == 1:  # diagonal block
                        nc.vector.tensor_mul(pt[:, 0:128], pt[:, 0:128], tri)
                    state[ui] = pt

                def emit_pvs(ui):
                    pt = state.pop(ui)
                    for kb, co, off, w in units[ui]:
                        nc.tensor.matmul(
                            zps[:, off:QC],
                            v_aug[:, kb, h],
                            pt[:, co : co + w],
                            start=(kb == 0),
                            stop=(kb == nblk - 1),
                        )

                nu = len(units)
                for ui in range(min(LEADU, nu)):
                    emit_scores(ui)
                for ui in range(nu):
                    emit_pvs(ui)
                    if ui + LEADU < nu:
                        emit_scores(ui + LEADU)

                # z = zps[0:64] / zps[64:128] via fast-inverse + 1 Newton step
                den_i = zps[64:128, :].bitcast(I32)
                x0 = recp.tile([64, QC], F32, tag="x0")
                nc.vector.scalar_tensor_tensor(
                    x0.bitcast(I32), magic_sb, 0, den_i,
                    mybir.AluOpType.bypass, mybir.AluOpType.subtract,
                )
                e = recp.tile([64, QC], F32, tag="e")
                nc.vector.tensor_mul(e, zps[64:128, :], x0)
                x1n = recp.tile([64, QC], F32, tag="x1n")
                nc.vector.scalar_tensor_tensor(
                    x1n, e, 2.0, x0,
                    mybir.AluOpType.subtract, mybir.AluOpType.mult,
                )
                nc.vector.scalar_tensor_tensor(
                    zTs[h][:, QC * qc : QC * (qc + 1)], zps[0:64, :], -1.0, x1n,
                    mybir.AluOpType.mult, mybir.AluOpType.mult,
                )

            def head_block(h):
                for qc in range(NQC):
                    if h == 0:
                        proj_v(tv[qc // 2], qc % 2, qc)
                        if qc == 0:
                            proj_q(0)
                    attention(h, qc)
                    if h == 0 and qc + 1 < NQC:
                        proj_q(qc + 1)
                nc.sync.dma_start(out=z_dram[h], in_=zTs[h])
                nc.gpsimd.collective_compute(
                    "AllGather",
                    mybir.AluOpType.bypass,
                    replica_groups=[[0, 1, 2, 3], [4, 5, 6, 7]],
                    ins=[z_dram[h].opt()],
                    outs=[zfh[h].opt()],
                )
                m0h = 64 * (h % 2)
                for j in range(4):
                    nc.gpsimd.dma_start(
                        out=zfull_sb[m0h : m0h + 64, 2 * j + h // 2], in_=zfh[h][j]
                    )

            def out_proj_pass(chunks, first):
                for qb in range(S // 128):
                    po_full = psB.tile([128, 1024], F32, tag="st")
                    po = po_full[:, 0:DO]
                    for ci, c in enumerate(chunks):
                        nc.tensor.matmul(
                            po,
                            zfull_sb[:, c, 128 * qb : 128 * (qb + 1)],
                            wo_sb[:, c],
                            start=(ci == 0),
                            stop=(ci == len(chunks) - 1),
                        )
                    if first:
                        nc.vector.tensor_add(ob_stage[:, qb], po, bo_sb)
                    else:
                        ob = obp.tile([128, DO], F32, tag="ob")
                        nc.vector.scalar_tensor_tensor(
                            ob, po, 1.0, ob_stage[:, qb],
                            mybir.AluOpType.bypass, mybir.AluOpType.add,
                        )
                        nc.sync.dma_start(
                            out=out.ap()[128 * qb : 128 * (qb + 1), :], in_=ob
                        )

            head_block(0)
            head_block(1)
            head_block(2)
            out_proj_pass([0, 2, 4, 6], first=True)
            head_block(3)
            out_proj_pass([1, 3, 5, 7], first=False)

    nc.finalize()
    return nc


_CACHE = {}


def kernel(**inputs):
    _install_ntff_hook()
    nc = _CACHE.get("nc")
    if nc is None:
        nc = build()
        _CACHE["nc"] = nc

    f16 = np.float16
    xs = {k: np.asarray(inputs[k], np.float32) for k in ("query_input", "key_input", "value_input")}
    W = {k: np.asarray(inputs[k], np.float32) for k in ("W_Q", "W_K", "W_V", "W_O")}
    b = {k: np.asarray(inputs[k], np.float32) for k in ("b_Q", "b_K", "b_V", "b_O")}
    # pre-transpose activations to [D, S] so device DMAs are contiguous
    xT16 = {k: [np.ascontiguousarray(v[g].T).astype(f16) for g in range(B)] for k, v in xs.items()}
    # pre-arrange projection weights to [D, he] per head group
    wd = {}
    for k in ("W_Q", "W_K", "W_V"):
        wd[k] = [
            np.ascontiguousarray(
                W[k][4 * r : 4 * (r + 1)].transpose(1, 0, 2).reshape(D, HPC * DH)
            ).astype(f16)
            for r in range(4)
        ]
    wo_full = W["W_O"].reshape(H * DH, D)
    wo_slices = [np.ascontiguousarray(wo_full[:, DO * r : DO * (r + 1)]).astype(f16) for r in range(4)]
    bo_slices = [np.ascontiguousarray(b["b_O"][DO * r : DO * (r + 1)]) for r in range(4)]

    in_maps = []
    for i in range(N_CORES):
        g, r = i // 4, i % 4
        in_maps.append(
            {
                "xq": xT16["query_input"][g],
                "xk": xT16["key_input"][g],
                "xv": xT16["value_input"][g],
                "wq": wd["W_Q"][r],
                "wk": wd["W_K"][r],
                "wv": wd["W_V"][r],
                "wo": wo_slices[r],
                "bq": np.ascontiguousarray(b["b_Q"][4 * r : 4 * (r + 1)]),
                "bk": np.ascontiguousarray(b["b_K"][4 * r : 4 * (r + 1)]),
                "bv": np.ascontiguousarray(b["b_V"][4 * r : 4 * (r + 1)]),
                "bo": bo_slices[r],
            }
        )

    res = run_bass_kernel_spmd(nc, in_maps, core_ids=list(range(N_CORES)))
    if os.environ.get("KERNEL_PRINT_EXEC"):
        print(f"HW exec time: {res.exec_time_ns} ns")
    outs = []
    for g in range(B):
        outs.append(
            np.concatenate([res.results[4 * g + r]["out"] for r in range(4)], axis=1)
        )
    return np.stack(outs, axis=0).astype(np.float32)
